# revision 1
# baseline (speedup 1.0000x reference)
"""Trainium2 Bass kernel: causal multi-head attention with interleaved RoPE.

Problem shapes (hardcoded): x [2, 2048, 1024], 16 heads of dk=64.
Sharding: 8 cores = 2 batches x 4 head-groups (4 heads each). Each core
computes its head-slice Q/K/V projections, RoPE, causal attention, and a
partial output through its Wo row-slice; the host sums the 4 partials per
batch and adds bo.

RoPE trick: attention scores are invariant to any permutation of the dk
axis applied to both Q and K, so the Wq/Wk columns are permuted on the host
into a "quadrant half-split" layout where each rotation pair partner sits
exactly 16 partitions away inside the same 32-partition quadrant. The DVE
stream_shuffle (a per-quadrant 32-way permute) then produces the swapped
operand, and RoPE becomes: rot = q * cosT + shuffle(q) * sinT with
host-precomputed tables (sinT carries the sign).
"""

import os
from contextlib import ExitStack

import numpy as np

import concourse.bass as bass
import concourse.mybir as mybir
import concourse.tile as tile

B, S, D, H = 2, 2048, 1024, 16
DK = D // H  # 64
HG = 4  # heads per core
NCOLS = HG * DK  # 256 columns of the projection per core
THETA = 10000.0
SCALE = 1.0 / float(np.sqrt(DK))
N_CORES = 8

F32 = mybir.dt.float32
F32R = mybir.dt.float32r

# matmul operand dtype: float32r (= TF32, 10-bit mantissa) streams 1 col/cycle
# on the PE vs 4 for float32. Operands must be *rounded* to TF32: DMA-fed
# tensors are pre-rounded on the host and declared float32r; on-chip operand
# producers write float32r directly. Numerics validated in test.py.
USE_F32R = os.environ.get("KERNEL_F32", "0") != "1"
MMDT = F32R if USE_F32R else F32


def round_tf32(a):
    """Round fp32 array to TF32 (RNE to 10-bit mantissa)."""
    if not USE_F32R:
        return np.ascontiguousarray(a, dtype=np.float32)
    u = np.ascontiguousarray(a, dtype=np.float32).view(np.uint32).copy()
    u += 0x0FFF + ((u >> 13) & 1)
    u &= np.uint32(0xFFFFE000)
    return u.view(np.float32)


# ---------------------------------------------------------------------------
# host-side prep
# ---------------------------------------------------------------------------

def _rope_perm():
    """Within-head column permutation pi: new row r -> original dk index."""
    perm = np.empty(DK, dtype=np.int64)
    for r in range(DK):
        q, m = divmod(r, 32)
        if m < 16:
            perm[r] = 2 * (16 * q + m)
        else:
            perm[r] = 2 * (16 * q + m - 16) + 1
    return perm


_PERM = _rope_perm()
SHUF_MASK = list(range(16, 32)) + list(range(16))  # swap 16-halves per quadrant


def _rope_tables(pos):
    """cosT/sinT [128, S] fp32 for the permuted layout. pos: [S] int."""
    inv_freq = (np.float32(THETA) ** (-(np.arange(0, DK, 2, dtype=np.float32) / np.float32(DK))))  # [32]
    ang = pos.astype(np.float32)[:, None] * inv_freq[None, :]  # [S, 32]
    cos = np.cos(ang)  # [S, 32]
    sin = np.sin(ang)
    cosT = np.empty((128, S), dtype=np.float32)
    sinT = np.empty((128, S), dtype=np.float32)
    for p in range(128):
        r = p % DK
        q, m = divmod(r, 32)
        if m < 16:
            i = 16 * q + m
            sgn = -1.0
        else:
            i = 16 * q + m - 16
            sgn = 1.0
        cosT[p] = cos[:, i]
        sinT[p] = np.float32(sgn) * sin[:, i]
    return cosT, sinT


def make_core_inputs(x, token_position, Wq, bq, Wk, bk, Wv, bv, Wo, bo):
    """Build the 8 per-core input maps."""
    x = np.asarray(x, dtype=np.float32)
    token_position = np.asarray(token_position)
    Wq, Wk, Wv, Wo = (np.asarray(w, dtype=np.float32) for w in (Wq, Wk, Wv, Wo))
    bq, bk, bv = (np.asarray(b_, dtype=np.float32) for b_ in (bq, bk, bv))

    in_maps = []
    tables = {}
    for c in range(N_CORES):
        b, hg = divmod(c, HG)
        heads = range(HG * hg, HG * hg + HG)
        # permuted q/k column indices for this core's heads
        cols_qk = np.concatenate([DK * h + _PERM for h in heads])
        cols_v = np.arange(NCOLS * hg, NCOLS * hg + NCOLS)
        if b not in tables:
            tables[b] = _rope_tables(np.asarray(token_position[b]))
        cosT, sinT = tables[b]
        wo_rows = Wo[cols_v, :]  # [256, 1024]
        in_maps.append({
            "xT": round_tf32(x[b].T),                               # [1024, 2048]
            "wq": round_tf32(Wq[:, cols_qk]),                       # [1024, 256]
            "wk": round_tf32(Wk[:, cols_qk]),
            "wv": round_tf32(Wv[:, cols_v]),
            "wo": round_tf32(wo_rows.reshape(HG, DK, D).transpose(1, 0, 2)),  # [64, 4, 1024]
            "bq": round_tf32(bq[cols_qk][None, :]),                 # [1, 256]
            "bk": round_tf32(bk[cols_qk][None, :]),
            "bv": round_tf32(bv[cols_v][None, :]),
            "ones_row": round_tf32(np.ones((1, 512), np.float32)),
            "onesc": round_tf32(np.ones((128, 64), np.float32)),
            "cosT": cosT,
            "sinT": sinT,
        })
    return in_maps


# ---------------------------------------------------------------------------
# device program
# ---------------------------------------------------------------------------

def build_program(with_bias=False):
    from concourse import bacc, library_config
    nc = bacc.Bacc("TRN2", debug=False)

    xT = nc.declare_dram_parameter("xT", [D, S], MMDT, isOutput=False).ap()
    wq = nc.declare_dram_parameter("wq", [D, NCOLS], MMDT, isOutput=False).ap()
    wk = nc.declare_dram_parameter("wk", [D, NCOLS], MMDT, isOutput=False).ap()
    wv = nc.declare_dram_parameter("wv", [D, NCOLS], MMDT, isOutput=False).ap()
    wo = nc.declare_dram_parameter("wo", [DK, HG, D], MMDT, isOutput=False).ap()
    bq = nc.declare_dram_parameter("bq", [1, NCOLS], MMDT, isOutput=False).ap()
    bk = nc.declare_dram_parameter("bk", [1, NCOLS], MMDT, isOutput=False).ap()
    bv = nc.declare_dram_parameter("bv", [1, NCOLS], MMDT, isOutput=False).ap()
    ones_row_d = nc.declare_dram_parameter("ones_row", [1, 512], MMDT, isOutput=False).ap()
    onesc_d = nc.declare_dram_parameter("onesc", [128, DK], MMDT, isOutput=False).ap()
    cosT = nc.declare_dram_parameter("cosT", [128, S], F32, isOutput=False).ap()
    sinT = nc.declare_dram_parameter("sinT", [128, S], F32, isOutput=False).ap()
    out = nc.declare_dram_parameter("out", [S, D], F32, isOutput=True).ap()

    SB = 512            # sq block width
    NSB = S // SB       # 4
    NST = S // 128      # 16 key tiles / V tiles
    NDC = D // 128      # 8 contraction chunks
    GW = 2              # key tiles per score-psum group

    with tile.TileContext(nc) as tc, ExitStack() as ctx:
        nc.gpsimd.load_library(library_config.proxy)
        const = ctx.enter_context(tc.tile_pool(name="const", bufs=1))
        sbig = ctx.enter_context(tc.tile_pool(name="sbig", bufs=1))
        xts = ctx.enter_context(tc.tile_pool(name="xts", bufs=4))
        rtmp = ctx.enter_context(tc.tile_pool(name="rtmp", bufs=2))
        epool = ctx.enter_context(tc.tile_pool(name="epool", bufs=3))
        npool = ctx.enter_context(tc.tile_pool(name="npool", bufs=3))
        opool = ctx.enter_context(tc.tile_pool(name="opool", bufs=2))

        # --- constants / weights resident in SBUF (per-dc tiles: finer deps,
        # so the first projection matmuls start after ~128KB of DMA)
        wq_sb = [const.tile([128, NCOLS], MMDT, tag=f"wq{dc}", name=f"wq{dc}")
                 for dc in range(NDC)]
        wk_sb = [const.tile([128, NCOLS], MMDT, tag=f"wk{dc}", name=f"wk{dc}")
                 for dc in range(NDC)]
        wv_sb = [const.tile([128, NCOLS], MMDT, tag=f"wv{dc}", name=f"wv{dc}")
                 for dc in range(NDC)]
        for dc in range(NDC):
            nc.sync.dma_start(wq_sb[dc][:], wq[128 * dc:128 * dc + 128, :])
            nc.sync.dma_start(wk_sb[dc][:], wk[128 * dc:128 * dc + 128, :])
        cos_sb = const.tile([128, S], F32, tag="cos")
        sin_sb = const.tile([128, S], F32, tag="sin")
        nc.sync.dma_start(cos_sb[:], cosT)
        nc.sync.dma_start(sin_sb[:], sinT)
        for dc in range(NDC):
            nc.sync.dma_start(wv_sb[dc][:], wv[128 * dc:128 * dc + 128, :])
        # wo padded to K=128 with zero rows 64-127: fp32r matmuls with K=64
        # stream at ~2 cycles/row (HW-measured), K=128 at 1 -- zero-padding
        # the contraction nearly halves scores/Wo PE time. DMA'd after the
        # critical-path inputs (only needed in the Wo phase).
        wo_sb = const.tile([128, HG, D], MMDT, tag="wo")
        nc.sync.dma_start(wo_sb[0:DK, :, :], wo)
        for a in range(2):
            nc.vector.tensor_scalar_mul(
                wo_sb[DK:128, 2 * a:2 * a + 2, :],
                sin_sb[DK:128, :].rearrange("p (a b) -> p a b", a=2), 0.0)
        if with_bias:
            bq_sb = const.tile([1, NCOLS], MMDT, tag="bq")
            bk_sb = const.tile([1, NCOLS], MMDT, tag="bk")
            bv_sb = const.tile([1, NCOLS], MMDT, tag="bv")
            nc.sync.dma_start(bq_sb[:], bq)
            nc.sync.dma_start(bk_sb[:], bk)
            nc.sync.dma_start(bv_sb[:], bv)
        ones_row = const.tile([1, SB], MMDT, tag="ones_row")
        nc.sync.dma_start(ones_row[:], ones_row_d)
        onesc_sb = const.tile([128, DK], MMDT, tag="onesc")
        nc.sync.dma_start(onesc_sb[:], onesc_d)

        # Q^T / K^T per (chunk, sq-block): chunk c holds heads {2c, 2c+1}
        qt = [[sbig.tile([128, SB], MMDT, tag=f"qt{c}_{sb}", name=f"qt{c}_{sb}")
               for sb in range(NSB)] for c in range(2)]
        # per-head K^T, zero-padded to 128 partitions (head data on its chunk
        # rows, the complementary 64 rows zeroed)
        kth = [[sbig.tile([128, SB], MMDT, tag=f"kh{h}_{sb}", name=f"kh{h}_{sb}")
                for sb in range(NSB)] for h in range(HG)]
        for h in range(HG):
            zrows = slice(DK, 128) if h % 2 == 0 else slice(0, DK)
            for sb in range(NSB):
                nc.vector.tensor_scalar_mul(kth[h][sb][zrows, :],
                                            cos_sb[zrows, 0:SB], 0.0)
        # V augmented with a ones column per head, per key tile. Head stride
        # padded 65 -> 68 columns so each head's lhsT starts 16B-aligned.
        AUGW = DK + 4
        vaug = [sbig.tile([128, HG * AUGW], MMDT, tag=f"va{st}", name=f"va{st}")
                for st in range(NST)]
        # unnormalized O^T per (head, sq-block), zero-padded to 128 rows
        ot = [[sbig.tile([128, SB], MMDT, tag=f"ot{h}_{j}", name=f"ot{h}_{j}")
               for j in range(NSB)] for h in range(HG)]
        for h in range(HG):
            for j in range(NSB):
                nc.vector.tensor_scalar_mul(ot[h][j][DK:128, :],
                                             cos_sb[DK:128, 0:SB], 0.0)

        # ------------------------------------------------------- projections
        with tc.tile_pool(name="pj_ps", bufs=4, space="PSUM") as pj_ps, \
             tc.tile_pool(name="pv_ps", bufs=4, space="PSUM") as pvp_ps:
            for sb in range(NSB):
                ss = slice(SB * sb, SB * sb + SB)
                xt_t = []
                for dc in range(NDC):
                    t = xts.tile([128, SB], MMDT, tag="xt")
                    nc.sync.dma_start(t[:], xT[128 * dc:128 * dc + 128, ss])
                    xt_t.append(t)
                for c in range(2):
                    ncol = slice(128 * c, 128 * c + 128)
                    for (w_sb, bname) in ((wq_sb, "bq"), (wk_sb, "bk")):
                        ps = pj_ps.tile([128, SB], F32, tag="qk")
                        for dc in range(NDC):
                            nc.tensor.matmul(ps[:], w_sb[dc][:, ncol], xt_t[dc][:],
                                             start=(dc == 0),
                                             stop=(dc == NDC - 1 and not with_bias))
                        if with_bias:
                            b_sb = bq_sb if bname == "bq" else bk_sb
                            nc.tensor.matmul(ps[:], b_sb[0:1, ncol], ones_row[0:1, :],
                                             start=False, stop=True)
                        # rope: dst = ps*cos + shuffle(ps)*sin
                        t_cos = rtmp.tile([128, SB], F32, tag="rc")
                        nc.vector.tensor_mul(t_cos[:], ps[:], cos_sb[:, ss])
                        t_shuf = rtmp.tile([128, SB], F32, tag="rs")
                        nc.vector.stream_shuffle(t_shuf[:], ps[:], SHUF_MASK)
                        t_sin = rtmp.tile([128, SB], F32, tag="rm")
                        nc.gpsimd.tensor_mul(t_sin[:], t_shuf[:], sin_sb[:, ss])
                        if bname == "bq":
                            nc.vector.tensor_add(qt[c][sb][:], t_cos[:], t_sin[:])
                        else:
                            nc.vector.tensor_add(kth[2 * c][sb][0:DK, :],
                                                 t_cos[0:DK, :], t_sin[0:DK, :])
                            nc.vector.tensor_add(kth[2 * c + 1][sb][DK:128, :],
                                                 t_cos[DK:128, :], t_sin[DK:128, :])
                for st4 in range(SB // 128):
                    st = (SB // 128) * sb + st4
                    ps = pvp_ps.tile([128, NCOLS], F32, tag="v")
                    for dc in range(NDC):
                        nc.tensor.matmul(ps[:], xt_t[dc][:, 128 * st4:128 * st4 + 128],
                                         wv_sb[dc][:],
                                         start=(dc == 0),
                                         stop=(dc == NDC - 1 and not with_bias))
                    if with_bias:
                        nc.tensor.matmul(ps[:], ones_row[0:1, 0:128], bv_sb[0:1, :],
                                         start=False, stop=True)
                    # scatter heads into the augmented layout; even heads get
                    # [V | ones], odd heads [ones | V] (so PV psum offset 63
                    # puts their output on partitions 64-127)
                    va = vaug[st][:].rearrange("p (h e) -> p h e", h=HG)
                    nc.vector.tensor_copy(va[:, :, 0:DK],
                                          ps[:].rearrange("p (h k) -> p h k", h=HG))
                    nc.vector.tensor_copy(va[:, :, DK], onesc_sb[:, 0:HG])

        # -------------------------------------------------------- attention
        # S^T layout: psum group = GW key tiles x one sq block; exp on ACT;
        # PV accumulates (V | ones) so row 64 is the softmax denominator.
        with tc.tile_pool(name="sc_ps", bufs=2, space="PSUM") as sc_ps, \
             tc.tile_pool(name="o_ps", bufs=2, space="PSUM") as o_ps, \
             tc.tile_pool(name="bc_ps", bufs=2, space="PSUM") as bc_ps:
            for j in range(NSB):
                sq = slice(SB * j, SB * j + SB)
                for h in range(HG):
                    c, half = divmod(h, 2)
                    rows = slice(DK * half, DK * half + DK)
                    pv = o_ps.tile([128, SB], F32, tag="pv")
                    ngrp = (4 * j + 4) // GW
                    for g in range(ngrp):
                        sc = sc_ps.tile([128, GW * SB], F32, tag="sc")
                        for t in range(GW):
                            i = GW * g + t
                            nc.tensor.matmul(
                                sc[:, SB * t:SB * t + SB],
                                kth[h][i // 4][:, 128 * (i % 4):128 * (i % 4) + 128],
                                qt[c][j][:],
                                start=True, stop=True)
                        e = epool.tile([128, GW * SB], MMDT, tag="e")
                        nc.scalar.activation(e[:], sc[:],
                                             mybir.ActivationFunctionType.Exp,
                                             scale=SCALE)
                        d0 = GW * g - 4 * j
                        if d0 + GW > 0:  # group touches the causal diagonal
                            ev = e[:].rearrange("p (t f) -> p t f", t=GW)
                            nc.gpsimd.affine_select(
                                out=ev, in_=ev,
                                compare_op=mybir.AluOpType.is_ge,
                                fill=0.0, base=-128 * d0,
                                pattern=[[-128, GW], [1, SB]],
                                channel_multiplier=-1)
                        for t in range(GW):
                            i = GW * g + t
                            lhs = vaug[i][:].rearrange("p (h e) -> p h e", h=HG)[:, h, 0:DK + 1]
                            nc.tensor.matmul(
                                pv[0:DK + 1, :], lhs, e[:, SB * t:SB * t + SB],
                                start=(g == 0 and t == 0),
                                stop=(g == ngrp - 1 and t == GW - 1))
                    # normalize: ot = pv[0:64] * broadcast(1/pv[64])
                    rec = npool.tile([128, SB], MMDT, tag="rec")
                    with nc.allow_low_precision(reason="denominator recip in tf32"):
                        nc.vector.reciprocal(rec[DK:DK + 1, :], pv[DK:DK + 1, :])
                    bcp = bc_ps.tile([DK, SB], F32, tag="bc")
                    nc.tensor.matmul(bcp[:], onesc_sb[DK:DK + 1, :],
                                     rec[DK:DK + 1, :], start=True, stop=True)
                    bc = npool.tile([DK, SB], F32, tag="bcs")
                    nc.vector.tensor_copy(bc[:], bcp[:])
                    nc.vector.tensor_mul(ot[h][j][0:DK, :], pv[0:DK, :], bc[:])

        # ------------------------------------------------- output projection
        with tc.tile_pool(name="wo_ps", bufs=4, space="PSUM") as wo_ps:
            for st in range(NST):
                rq = slice(128 * (st % 4), 128 * (st % 4) + 128)
                jb = st // 4
                for dc in range(2):
                    cols = slice(SB * dc, SB * dc + SB)
                    ps = wo_ps.tile([128, SB], F32, tag="wo")
                    for h in range(HG):
                        nc.tensor.matmul(ps[:], ot[h][jb][:, rq], wo_sb[:, h, cols],
                                         start=(h == 0), stop=(h == HG - 1))
                    o_sb = opool.tile([128, SB], F32, tag="osb")
                    if (st + dc) % 2 == 0:
                        nc.vector.tensor_copy(o_sb[:], ps[:])
                    else:
                        nc.scalar.copy(o_sb[:], ps[:])
                    nc.sync.dma_start(out[128 * st:128 * st + 128, cols], o_sb[:])

    nc.compile()
    return nc


_CACHED_NC = {}


def _get_program(with_bias=False):
    if with_bias not in _CACHED_NC:
        _CACHED_NC[with_bias] = build_program(with_bias=with_bias)
    return _CACHED_NC[with_bias]


# ---------------------------------------------------------------------------
# entry point
# ---------------------------------------------------------------------------

def kernel(x, token_position, Wq, bq, Wk, bk, Wv, bv, Wo, bo, _results=None):
    from concourse.bass_utils import run_bass_kernel_spmd

    in_maps = make_core_inputs(x, token_position, Wq, bq, Wk, bk, Wv, bv, Wo, bo)
    if _results is None:
        with_bias = any(float(np.abs(np.asarray(v)).max()) != 0.0
                        for v in (bq, bk, bv))
        nc = _get_program(with_bias=with_bias)
        res = run_bass_kernel_spmd(nc, in_maps, list(range(N_CORES)))
        _results = [res.results[i]["out"] for i in range(N_CORES)]
    bo = np.asarray(bo, dtype=np.float32)
    out = np.empty((B, S, D), dtype=np.float32)
    for b in range(B):
        acc = _results[HG * b].astype(np.float32)
        for hg in range(1, HG):
            acc = acc + _results[HG * b + hg]
        out[b] = acc + bo[None, :]
    return out



# revision 16
# speedup vs baseline: 1.1107x; 1.1107x over previous
"""Trainium2 Bass kernel: causal multi-head attention with interleaved RoPE.

Problem shapes (hardcoded): x [2, 2048, 1024], 16 heads of dk=64.
Sharding: 8 cores = 2 batches x 4 head-groups (4 heads each). Each core
computes its head-slice Q/K/V projections, RoPE, causal attention, and a
partial output through its Wo row-slice; the host sums the 4 partials per
batch and adds bo.

RoPE trick: attention scores are invariant to any permutation of the dk
axis applied to both Q and K, so the Wq/Wk columns are permuted on the host
into a "quadrant half-split" layout where each rotation pair partner sits
exactly 16 partitions away inside the same 32-partition quadrant. The DVE
stream_shuffle (a per-quadrant 32-way permute) then produces the swapped
operand, and RoPE becomes: rot = q * cosT + shuffle(q) * sinT with
host-precomputed tables (sinT carries the sign).

Schedule: one interleaved instruction stream. Projections for seq-block
j+1 and the Wo output projection for block j-1 are emitted as PE fillers
between the score/PV matmuls of block j's attention, so the Tensor engine
never drains (stays at max p-state) while the Scalar engine runs exp.
Head pairs share one ot tile (odd head's PV lands on PSUM partitions
63:128 via the [1|V] augmented-V layout) so Wo needs 2 full-K matmuls
per tile instead of 4 half-K ones.
"""

import os
from collections import deque
from contextlib import ExitStack

import numpy as np
import ml_dtypes

import concourse.bass as bass
import concourse.mybir as mybir
import concourse.tile as tile

B, S, D, H = 2, 2048, 1024, 16
DK = D // H  # 64
HG = 4  # heads per core
NCOLS = HG * DK  # 256 columns of the projection per core
THETA = 10000.0
SCALE = 1.0 / float(np.sqrt(DK))
N_CORES = 8

F32 = mybir.dt.float32
F32R = mybir.dt.float32r
BF16 = mybir.dt.bfloat16

SB = 512            # sq block width
NSB = S // SB       # 4
NST = S // 128      # 16 key tiles / V tiles
NDC = D // 128      # 8 contraction chunks
GW = 2              # key tiles per score-psum group
AUGW = DK + 8       # V head stride padded so each head's lhsT is 16B aligned

BF = ml_dtypes.bfloat16


def to_bf16(a):
    return np.ascontiguousarray(np.asarray(a, dtype=np.float32)).astype(BF)


# ---------------------------------------------------------------------------
# host-side prep
# ---------------------------------------------------------------------------

def _rope_perm():
    """Within-head column permutation pi: new row r -> original dk index."""
    perm = np.empty(DK, dtype=np.int64)
    for r in range(DK):
        q, m = divmod(r, 32)
        if m < 16:
            perm[r] = 2 * (16 * q + m)
        else:
            perm[r] = 2 * (16 * q + m - 16) + 1
    return perm


_PERM = _rope_perm()
SHUF_MASK = list(range(16, 32)) + list(range(16))  # swap 16-halves per quadrant


def _causal_masks():
    """keep(p, t, f) = f >= 128*d0 + 128*t + p for d0 in (0, 2)."""
    masks = []
    p_ = np.arange(128)[:, None, None]
    t_ = np.arange(GW)[None, :, None]
    f_ = np.arange(SB)[None, None, :]
    for d0 in (0, 2):
        keep = (f_ >= 128 * d0 + 128 * t_ + p_)
        masks.append(to_bf16(keep.astype(np.float32).reshape(128, GW * SB)))
    return masks


_CAUSAL_MASKS = _causal_masks()


def _rope_tables(pos):
    """cosT/sinT [128, S] fp32 for the permuted layout. pos: [S] int."""
    inv_freq = (np.float32(THETA) ** (-(np.arange(0, DK, 2, dtype=np.float32) / np.float32(DK))))  # [32]
    ang = pos.astype(np.float32)[:, None] * inv_freq[None, :]  # [S, 32]
    cos = np.cos(ang)  # [S, 32]
    sin = np.sin(ang)
    cosT = np.empty((128, S), dtype=np.float32)
    sinT = np.empty((128, S), dtype=np.float32)
    for p in range(128):
        r = p % DK
        q, m = divmod(r, 32)
        if m < 16:
            i = 16 * q + m
            sgn = -1.0
        else:
            i = 16 * q + m - 16
            sgn = 1.0
        cosT[p] = cos[:, i]
        sinT[p] = np.float32(sgn) * sin[:, i]
    return cosT, sinT


def make_core_inputs(x, token_position, Wq, bq, Wk, bk, Wv, bv, Wo, bo):
    """Build the 8 per-core input maps."""
    x = np.asarray(x, dtype=np.float32)
    token_position = np.asarray(token_position)
    Wq, Wk, Wv, Wo = (np.asarray(w, dtype=np.float32) for w in (Wq, Wk, Wv, Wo))
    bq, bk, bv = (np.asarray(b_, dtype=np.float32) for b_ in (bq, bk, bv))

    in_maps = []
    tables = {}
    for c in range(N_CORES):
        b, hg = divmod(c, HG)
        heads = range(HG * hg, HG * hg + HG)
        # permuted q/k column indices for this core's heads
        cols_qk = np.concatenate([DK * h + _PERM for h in heads])
        cols_v = np.arange(NCOLS * hg, NCOLS * hg + NCOLS)
        if b not in tables:
            tables[b] = _rope_tables(np.asarray(token_position[b]))
        cosT, sinT = tables[b]
        wo_rows = Wo[cols_v, :]  # [256, 1024]
        in_maps.append({
            "xT": to_bf16(x[b].T),                              # [1024, 2048]
            "wq": to_bf16(Wq[:, cols_qk]),                      # [1024, 256]
            "wk": to_bf16(Wk[:, cols_qk]),
            "wv": to_bf16(Wv[:, cols_v]),
            "wo": to_bf16(wo_rows.reshape(HG, DK, D).transpose(1, 0, 2)),  # [64, 4, 1024]
            "bq": to_bf16(bq[cols_qk][None, :]),                # [1, 256]
            "bk": to_bf16(bk[cols_qk][None, :]),
            "bv": to_bf16(bv[cols_v][None, :]),
            "ones_row": to_bf16(np.ones((1, SB), np.float32)),
            "onesc": to_bf16(np.ones((128, DK), np.float32)),
            "onesr": np.ones((128, DK), np.float32),
            "maskA": _CAUSAL_MASKS[0],
            "maskB": _CAUSAL_MASKS[1],
            "cosT": cosT,
            "sinT": sinT,
        })
    return in_maps


# ---------------------------------------------------------------------------
# device program
# ---------------------------------------------------------------------------

def build_program(with_bias=False):
    from concourse import bacc, library_config
    nc = bacc.Bacc("TRN2", debug=False)

    xT = nc.declare_dram_parameter("xT", [D, S], BF16, isOutput=False).ap()
    wq = nc.declare_dram_parameter("wq", [D, NCOLS], BF16, isOutput=False).ap()
    wk = nc.declare_dram_parameter("wk", [D, NCOLS], BF16, isOutput=False).ap()
    wv = nc.declare_dram_parameter("wv", [D, NCOLS], BF16, isOutput=False).ap()
    wo = nc.declare_dram_parameter("wo", [DK, HG, D], BF16, isOutput=False).ap()
    bq = nc.declare_dram_parameter("bq", [1, NCOLS], BF16, isOutput=False).ap()
    bk = nc.declare_dram_parameter("bk", [1, NCOLS], BF16, isOutput=False).ap()
    bv = nc.declare_dram_parameter("bv", [1, NCOLS], BF16, isOutput=False).ap()
    ones_row_d = nc.declare_dram_parameter("ones_row", [1, SB], BF16, isOutput=False).ap()
    onesc_d = nc.declare_dram_parameter("onesc", [128, DK], BF16, isOutput=False).ap()
    onesr_d = nc.declare_dram_parameter("onesr", [128, DK], F32R, isOutput=False).ap()
    maskA_d = nc.declare_dram_parameter("maskA", [128, GW * SB], BF16, isOutput=False).ap()
    maskB_d = nc.declare_dram_parameter("maskB", [128, GW * SB], BF16, isOutput=False).ap()
    cosT = nc.declare_dram_parameter("cosT", [128, S], F32, isOutput=False).ap()
    sinT = nc.declare_dram_parameter("sinT", [128, S], F32, isOutput=False).ap()
    out = nc.declare_dram_parameter("out", [S, D], F32, isOutput=True).ap()
    debug_dump = os.environ.get("KERNEL_DEBUG_DUMP", "0") == "1"
    if debug_dump:
        dbg_qt = nc.declare_dram_parameter("dbg_qt", [128, SB], F32, isOutput=True).ap()
        dbg_kh = nc.declare_dram_parameter("dbg_kh", [128, SB], F32, isOutput=True).ap()
        dbg_va = nc.declare_dram_parameter("dbg_va", [128, HG * (DK + 8)], F32, isOutput=True).ap()
        dbg_ot = nc.declare_dram_parameter("dbg_ot", [NSB, HG, DK, SB], F32, isOutput=True).ap()
        dbg_den = nc.declare_dram_parameter("dbg_den", [NSB, HG, 2, SB], F32, isOutput=True).ap()

    with tile.TileContext(nc) as tc, ExitStack() as ctx:
        nc.gpsimd.load_library(library_config.proxy)
        const = ctx.enter_context(tc.tile_pool(name="const", bufs=1))
        sbig = ctx.enter_context(tc.tile_pool(name="sbig", bufs=1))
        rtmp = ctx.enter_context(tc.tile_pool(name="rtmp", bufs=2))
        epool = ctx.enter_context(tc.tile_pool(name="epool", bufs=3))
        npool = ctx.enter_context(tc.tile_pool(name="npool", bufs=2))
        opool = ctx.enter_context(tc.tile_pool(name="opool", bufs=2))
        sc_ps = ctx.enter_context(tc.tile_pool(name="sc_ps", bufs=2, space="PSUM"))
        pv_ps = ctx.enter_context(tc.tile_pool(name="pv_ps", bufs=2, space="PSUM"))
        mm_ps = ctx.enter_context(tc.tile_pool(name="mm_ps", bufs=2, space="PSUM"))

        # --- static SBUF tiles
        wq_sb = [const.tile([128, NCOLS], BF16, tag=f"wq{dc}", name=f"wq{dc}")
                 for dc in range(NDC)]
        wk_sb = [const.tile([128, NCOLS], BF16, tag=f"wk{dc}", name=f"wk{dc}")
                 for dc in range(NDC)]
        wv_sb = [const.tile([128, NCOLS], BF16, tag=f"wv{dc}", name=f"wv{dc}")
                 for dc in range(NDC)]
        cos_sb = const.tile([128, S], F32, tag="cos")
        sin_sb = const.tile([128, S], F32, tag="sin")
        wo_sb = const.tile([DK, HG, D], BF16, tag="wo")
        onesc_sb = const.tile([128, DK], BF16, tag="onesc")
        onesr_sb = const.tile([128, DK], F32R, tag="onesr")
        mask_sb = [const.tile([128, GW * SB], BF16, tag=f"mask{i}", name=f"mask{i}")
                   for i in range(2)]
        if with_bias:
            bq_sb = const.tile([1, NCOLS], BF16, tag="bq")
            bk_sb = const.tile([1, NCOLS], BF16, tag="bk")
            bv_sb = const.tile([1, NCOLS], BF16, tag="bv")
            ones_row = const.tile([1, SB], BF16, tag="ones_row")
        xt = [[sbig.tile([128, SB], BF16, tag=f"xt{sb}_{dc}", name=f"xt{sb}_{dc}")
               for dc in range(NDC)] for sb in range(NSB)]
        # Q^T / K^T per (chunk, sq-block): chunk c holds heads {2c, 2c+1}
        # stacked on partitions (head 2c rows 0:64, head 2c+1 rows 64:128)
        qt = [[sbig.tile([128, SB], BF16, tag=f"qt{c}_{sb}", name=f"qt{c}_{sb}")
               for sb in range(NSB)] for c in range(2)]
        kth = [[sbig.tile([128, SB], BF16, tag=f"kh{c}_{sb}", name=f"kh{c}_{sb}")
                for sb in range(NSB)] for c in range(2)]
        # V augmented per key tile, every head [V(64) | one] so PV row 64
        # accumulates the softmax denominator
        vaug = [sbig.tile([128, HG * AUGW], BF16, tag=f"va{st}", name=f"va{st}")
                for st in range(NST)]
        # normalized O^T per (head, sq-block), rows 0:64
        ot = [[sbig.tile([DK, SB], BF16, tag=f"ot{h}_{j}", name=f"ot{h}_{j}")
               for j in range(NSB)] for h in range(HG)]

        # --- DMAs, critical-path first
        for dc in range(NDC):
            nc.sync.dma_start(wq_sb[dc][:], wq[128 * dc:128 * dc + 128, :])
            nc.sync.dma_start(xt[0][dc][:], xT[128 * dc:128 * dc + 128, 0:SB])
        nc.sync.dma_start(cos_sb[:], cosT)
        nc.sync.dma_start(sin_sb[:], sinT)
        for dc in range(NDC):
            nc.sync.dma_start(wk_sb[dc][:], wk[128 * dc:128 * dc + 128, :])
        for dc in range(NDC):
            nc.sync.dma_start(wv_sb[dc][:], wv[128 * dc:128 * dc + 128, :])
        nc.sync.dma_start(onesc_sb[:], onesc_d)
        nc.sync.dma_start(onesr_sb[:], onesr_d)
        nc.sync.dma_start(mask_sb[0][:], maskA_d)
        nc.sync.dma_start(mask_sb[1][:], maskB_d)
        for dc in range(NDC):
            nc.sync.dma_start(xt[1][dc][:], xT[128 * dc:128 * dc + 128, SB:2 * SB])
        nc.sync.dma_start(wo_sb[:], wo)
        if with_bias:
            nc.sync.dma_start(bq_sb[:], bq)
            nc.sync.dma_start(bk_sb[:], bk)
            nc.sync.dma_start(bv_sb[:], bv)
            nc.sync.dma_start(ones_row[:], ones_row_d)
        for sb in (2, 3):
            for dc in range(NDC):
                nc.sync.dma_start(xt[sb][dc][:],
                                  xT[128 * dc:128 * dc + 128, SB * sb:SB * sb + SB])

        # --- emission helpers -------------------------------------------
        def emit_qk_chunk(sb, c, wname):
            """Projection chunk c of Q or K for sq block sb, incl. RoPE."""
            w_sb = wq_sb if wname == "q" else wk_sb
            ss = slice(SB * sb, SB * sb + SB)
            ncol = slice(128 * c, 128 * c + 128)
            ps = mm_ps.tile([128, SB], F32, tag="mm", name="ps_qk")
            for dc in range(NDC):
                nc.tensor.matmul(ps[:], w_sb[dc][:, ncol], xt[sb][dc][:],
                                 start=(dc == 0),
                                 stop=(dc == NDC - 1 and not with_bias))
            if with_bias:
                b_sb = bq_sb if wname == "q" else bk_sb
                nc.tensor.matmul(ps[:], b_sb[0:1, ncol], ones_row[0:1, :],
                                 start=False, stop=True)
            # rope: dst = ps*cos + shuffle(ps)*sin
            t_cos = rtmp.tile([128, SB], F32, tag="rc", name="t_cos")
            nc.vector.tensor_mul(t_cos[:], ps[:], cos_sb[:, ss])
            t_shuf = rtmp.tile([128, SB], F32, tag="rs", name="t_shuf")
            nc.vector.stream_shuffle(t_shuf[:], ps[:], SHUF_MASK)
            t_sin = rtmp.tile([128, SB], F32, tag="rm", name="t_sin")
            nc.gpsimd.tensor_mul(t_sin[:], t_shuf[:], sin_sb[:, ss])
            dst = qt[c][sb] if wname == "q" else kth[c][sb]
            nc.vector.tensor_add(dst[:], t_cos[:], t_sin[:])

        def emit_v_st(sb, st4):
            """V projection for one 128-seq tile, scattered into vaug."""
            st = 4 * sb + st4
            ps = mm_ps.tile([128, SB], F32, tag="mm", name="ps_v")
            for dc in range(NDC):
                nc.tensor.matmul(ps[:, 0:NCOLS],
                                 xt[sb][dc][:, 128 * st4:128 * st4 + 128],
                                 wv_sb[dc][:],
                                 start=(dc == 0),
                                 stop=(dc == NDC - 1 and not with_bias))
            if with_bias:
                nc.tensor.matmul(ps[:, 0:NCOLS], ones_row[0:1, 0:128],
                                 bv_sb[0:1, :], start=False, stop=True)
            va = vaug[st][:].rearrange("p (h e) -> p h e", h=HG)
            psv = ps[:, 0:NCOLS].rearrange("p (h k) -> p h k", h=HG)
            nc.vector.tensor_copy(va[:, :, 0:DK], psv[:, :, :])
            nc.vector.tensor_copy(va[:, :, DK], onesc_sb[:, 0:HG])

        wo_copy_tick = [0]

        def emit_wo(st, dc):
            """Output projection for one (128-seq, 512-dmodel) tile."""
            jb = st // 4
            rq = slice(128 * (st % 4), 128 * (st % 4) + 128)
            cols = slice(SB * dc, SB * dc + SB)
            ps = mm_ps.tile([128, SB], F32, tag="mm", name="ps_wo")
            for h in range(HG):
                nc.tensor.matmul(ps[:], ot[h][jb][:, rq], wo_sb[:, h, cols],
                                 start=(h == 0), stop=(h == HG - 1))
            o_sb = opool.tile([128, SB], F32, tag="osb", name="o_sb")
            if wo_copy_tick[0] % 2 == 0:
                nc.vector.tensor_copy(o_sb[:], ps[:])
            else:
                nc.scalar.copy(o_sb[:], ps[:])
            wo_copy_tick[0] += 1
            nc.sync.dma_start(out[128 * st:128 * st + 128, cols], o_sb[:])

        def emit_norm(pv_t, rec, h, j):
            """ot[h][j] = pv V-rows * broadcast(1/denominator)."""
            bcp = mm_ps.tile([128, SB], F32, tag="mm", name="bcp")
            nc.tensor.matmul(bcp[0:DK, :], onesr_sb[DK:DK + 1, :],
                             rec[DK:DK + 1, :],
                             start=True, stop=True)
            bc = npool.tile([128, SB], F32, tag="bcs", name="bc")
            nc.vector.tensor_copy(bc[0:DK, :], bcp[0:DK, :])
            nc.vector.tensor_mul(ot[h][j][:], pv_t[0:DK, :], bc[0:DK, :])

        # --- projections for block 0 (nothing to overlap them with)
        emit_qk_chunk(0, 0, "q")
        emit_qk_chunk(0, 0, "k")
        emit_qk_chunk(0, 1, "q")
        emit_qk_chunk(0, 1, "k")
        for st4 in range(4):
            emit_v_st(0, st4)

        # --- main interleaved stream ------------------------------------
        seq = os.environ.get("KERNEL_SEQ", "0") == "1"
        if seq:
            for nb in range(1, NSB):
                emit_qk_chunk(nb, 0, "q")
                emit_qk_chunk(nb, 0, "k")
                emit_qk_chunk(nb, 1, "q")
                emit_qk_chunk(nb, 1, "k")
                for st4 in range(4):
                    emit_v_st(nb, st4)
        fillers = deque()
        pending_norm = deque()  # closures, flushed after the next sc group
        for j in range(NSB):
            if not seq and j < NSB - 1:
                nb = j + 1
                fillers.append(lambda nb=nb: emit_qk_chunk(nb, 0, "q"))
                fillers.append(lambda nb=nb: emit_qk_chunk(nb, 0, "k"))
                fillers.append(lambda nb=nb: emit_qk_chunk(nb, 1, "q"))
                fillers.append(lambda nb=nb: emit_qk_chunk(nb, 1, "k"))
                for st4 in range(4):
                    fillers.append(lambda nb=nb, st4=st4: emit_v_st(nb, st4))
            if not seq and j > 0:
                for st in range(4 * (j - 1), 4 * j):
                    for dc in range(2):
                        fillers.append(lambda st=st, dc=dc: emit_wo(st, dc))

            for h in range(HG):
                c, parity = divmod(h, 2)
                rows = slice(DK * parity, DK * parity + DK)
                pv = pv_ps.tile([128, SB], F32, tag="pv", name="pv")
                pv_out = pv[0:DK + 1, :]
                ngrp = 2 * (j + 1)
                for g in range(ngrp):
                    sc = sc_ps.tile([128, GW * SB], F32, tag="sc", name="sc")
                    for t in range(GW):
                        i = GW * g + t
                        nc.tensor.matmul(
                            sc[:, SB * t:SB * t + SB],
                            kth[c][i // 4][rows, 128 * (i % 4):128 * (i % 4) + 128],
                            qt[c][j][rows, :],
                            start=True, stop=True)
                    while pending_norm:
                        pending_norm.popleft()()
                    e = epool.tile([128, GW * SB], BF16, tag="e", name="e")
                    nc.scalar.activation(e[:], sc[:],
                                         mybir.ActivationFunctionType.Exp,
                                         scale=SCALE)
                    d0 = GW * g - 4 * j
                    if d0 + GW > 0:  # group touches the causal diagonal
                        nc.vector.tensor_mul(e[:], e[:],
                                             mask_sb[d0 // 2][:])
                    if fillers:
                        fillers.popleft()()
                    for t in range(GW):
                        i = GW * g + t
                        lhs = vaug[i][:].rearrange("p (h e) -> p h e", h=HG)[:, h, 0:DK + 1]
                        nc.tensor.matmul(
                            pv_out, lhs, e[:, SB * t:SB * t + SB],
                            start=(g == 0 and t == 0),
                            stop=(g == ngrp - 1 and t == GW - 1))
                # denominator reciprocal now; broadcast+normalize deferred so
                # the recip latency hides under the next head's first group
                rec = npool.tile([128, SB], F32R, tag="rec", name="rec")
                if debug_dump:
                    dd = opool.tile([128, SB], F32, tag="dbg", name="dd")
                    nc.vector.tensor_copy(dd[DK:DK + 1, :], pv[DK:DK + 1, :])
                    nc.sync.dma_start(dbg_den[j, h, 0], dd[DK:DK + 1, :])
                with nc.allow_low_precision(reason="softmax denom recip tf32"):
                    nc.vector.reciprocal(rec[DK:DK + 1, :], pv[DK:DK + 1, :])
                if debug_dump:
                    nc.sync.dma_start(dbg_den[j, h, 1], rec[DK:DK + 1, :])
                pending_norm.append(
                    lambda pv_t=pv, rec=rec, hh=h, jj=j:
                        emit_norm(pv_t, rec, hh, jj))

        # --- tail: last normalize + Wo for block 3
        while pending_norm:
            pending_norm.popleft()()
        while fillers:
            fillers.popleft()()
        wo_start = 0 if seq else 4 * (NSB - 1)
        for st in range(wo_start, 4 * NSB):
            for dc in range(2):
                emit_wo(st, dc)

        if debug_dump:
            dq = opool.tile([128, SB], F32, tag="dbg", name="dq")
            nc.vector.tensor_copy(dq[:], qt[0][1][:])
            nc.sync.dma_start(dbg_qt, dq[:])
            dk_ = opool.tile([128, SB], F32, tag="dbg", name="dk_")
            nc.vector.tensor_copy(dk_[:], kth[0][1][:])
            nc.sync.dma_start(dbg_kh, dk_[:])
            dv = opool.tile([128, HG * AUGW], F32, tag="dbgv", name="dv")
            nc.vector.tensor_copy(dv[:], vaug[4][:])
            nc.sync.dma_start(dbg_va, dv[:])
            for jj in range(NSB):
                for hh in range(HG):
                    do = opool.tile([128, SB], F32, tag="dbg", name="do")
                    nc.vector.tensor_copy(do[0:DK, :], ot[hh][jj][:])
                    nc.sync.dma_start(dbg_ot[jj, hh], do[0:DK, :])

    nc.compile()
    return nc


_CACHED_NC = {}


def _get_program(with_bias=False):
    if with_bias not in _CACHED_NC:
        _CACHED_NC[with_bias] = build_program(with_bias=with_bias)
    return _CACHED_NC[with_bias]


# ---------------------------------------------------------------------------
# entry point
# ---------------------------------------------------------------------------

def kernel(x, token_position, Wq, bq, Wk, bk, Wv, bv, Wo, bo, _results=None):
    from concourse.bass_utils import run_bass_kernel_spmd

    in_maps = make_core_inputs(x, token_position, Wq, bq, Wk, bk, Wv, bv, Wo, bo)
    if _results is None:
        with_bias = any(float(np.abs(np.asarray(v)).max()) != 0.0
                        for v in (bq, bk, bv))
        nc = _get_program(with_bias=with_bias)
        res = run_bass_kernel_spmd(nc, in_maps, list(range(N_CORES)))
        _results = [res.results[i]["out"] for i in range(N_CORES)]
    bo = np.asarray(bo, dtype=np.float32)
    out = np.empty((B, S, D), dtype=np.float32)
    for b in range(B):
        acc = _results[HG * b].astype(np.float32)
        for hg in range(1, HG):
            acc = acc + _results[HG * b + hg]
        out[b] = acc + bo[None, :]
    return out


# revision 17
# speedup vs baseline: 1.1502x; 1.0356x over previous
"""Trainium2 Bass kernel: causal multi-head attention with interleaved RoPE.

Problem shapes (hardcoded): x [2, 2048, 1024], 16 heads of dk=64.
Sharding: 8 cores = 2 batches x 4 head-groups (4 heads each). Each core
computes its head-slice Q/K/V projections, RoPE, causal attention, and a
partial output through its Wo row-slice; the host sums the 4 partials per
batch and adds bo.

RoPE trick: attention scores are invariant to any permutation of the dk
axis applied to both Q and K, so the Wq/Wk columns are permuted on the host
into a "quadrant half-split" layout where each rotation pair partner sits
exactly 16 partitions away inside the same 32-partition quadrant. The DVE
stream_shuffle (a per-quadrant 32-way permute) then produces the swapped
operand, and RoPE becomes: rot = q * cosT + shuffle(q) * sinT with
host-precomputed tables (sinT carries the sign).

Schedule: one interleaved instruction stream. Projections for seq-block
j+1 and the Wo output projection for block j-1 are emitted as PE fillers
between the score/PV matmuls of block j's attention, so the Tensor engine
never drains (stays at max p-state) while the Scalar engine runs exp.
Head pairs share one ot tile (odd head's PV lands on PSUM partitions
63:128 via the [1|V] augmented-V layout) so Wo needs 2 full-K matmuls
per tile instead of 4 half-K ones.
"""

import os
from collections import deque
from contextlib import ExitStack

import numpy as np
import ml_dtypes

import concourse.bass as bass
import concourse.mybir as mybir
import concourse.tile as tile

B, S, D, H = 2, 2048, 1024, 16
DK = D // H  # 64
HG = 4  # heads per core
NCOLS = HG * DK  # 256 columns of the projection per core
THETA = 10000.0
SCALE = 1.0 / float(np.sqrt(DK))
N_CORES = 8

F32 = mybir.dt.float32
F32R = mybir.dt.float32r
BF16 = mybir.dt.bfloat16

SB = 512            # sq block width
NSB = S // SB       # 4
NST = S // 128      # 16 key tiles / V tiles
NDC = D // 128      # 8 contraction chunks
GW = 2              # key tiles per score-psum group
AUGW = DK + 8       # V head stride padded so each head's lhsT is 16B aligned

BF = ml_dtypes.bfloat16


def to_bf16(a):
    return np.ascontiguousarray(np.asarray(a, dtype=np.float32)).astype(BF)


# ---------------------------------------------------------------------------
# host-side prep
# ---------------------------------------------------------------------------

def _rope_perm():
    """Within-head column permutation pi: new row r -> original dk index."""
    perm = np.empty(DK, dtype=np.int64)
    for r in range(DK):
        q, m = divmod(r, 32)
        if m < 16:
            perm[r] = 2 * (16 * q + m)
        else:
            perm[r] = 2 * (16 * q + m - 16) + 1
    return perm


_PERM = _rope_perm()
SHUF_MASK = list(range(16, 32)) + list(range(16))  # swap 16-halves per quadrant


def _causal_masks():
    """keep(p, t, f) = f >= 128*d0 + 128*t + p for d0 in (0, 2)."""
    masks = []
    p_ = np.arange(128)[:, None, None]
    t_ = np.arange(GW)[None, :, None]
    f_ = np.arange(SB)[None, None, :]
    for d0 in (0, 2):
        keep = (f_ >= 128 * d0 + 128 * t_ + p_)
        masks.append(to_bf16(keep.astype(np.float32).reshape(128, GW * SB)))
    return masks


_CAUSAL_MASKS = _causal_masks()


def _rope_tables(pos):
    """cosT/sinT [128, S] fp32 for the permuted layout. pos: [S] int."""
    inv_freq = (np.float32(THETA) ** (-(np.arange(0, DK, 2, dtype=np.float32) / np.float32(DK))))  # [32]
    ang = pos.astype(np.float32)[:, None] * inv_freq[None, :]  # [S, 32]
    cos = np.cos(ang)  # [S, 32]
    sin = np.sin(ang)
    cosT = np.empty((128, S), dtype=np.float32)
    sinT = np.empty((128, S), dtype=np.float32)
    for p in range(128):
        r = p % DK
        q, m = divmod(r, 32)
        if m < 16:
            i = 16 * q + m
            sgn = -1.0
        else:
            i = 16 * q + m - 16
            sgn = 1.0
        cosT[p] = cos[:, i]
        sinT[p] = np.float32(sgn) * sin[:, i]
    return cosT, sinT


def make_core_inputs(x, token_position, Wq, bq, Wk, bk, Wv, bv, Wo, bo):
    """Build the 8 per-core input maps."""
    x = np.asarray(x, dtype=np.float32)
    token_position = np.asarray(token_position)
    Wq, Wk, Wv, Wo = (np.asarray(w, dtype=np.float32) for w in (Wq, Wk, Wv, Wo))
    bq, bk, bv = (np.asarray(b_, dtype=np.float32) for b_ in (bq, bk, bv))

    in_maps = []
    tables = {}
    for c in range(N_CORES):
        b, hg = divmod(c, HG)
        heads = range(HG * hg, HG * hg + HG)
        # permuted q/k column indices for this core's heads
        cols_qk = np.concatenate([DK * h + _PERM for h in heads])
        cols_v = np.arange(NCOLS * hg, NCOLS * hg + NCOLS)
        if b not in tables:
            tables[b] = _rope_tables(np.asarray(token_position[b]))
        cosT, sinT = tables[b]
        wo_rows = Wo[cols_v, :]  # [256, 1024]
        in_maps.append({
            "xT": to_bf16(x[b].T),                              # [1024, 2048]
            "wq": to_bf16(Wq[:, cols_qk]),                      # [1024, 256]
            "wk": to_bf16(Wk[:, cols_qk]),
            "wv": to_bf16(Wv[:, cols_v]),
            "wo": to_bf16(wo_rows.reshape(HG, DK, D).transpose(1, 0, 2)),  # [64, 4, 1024]
            "bq": to_bf16(bq[cols_qk][None, :]),                # [1, 256]
            "bk": to_bf16(bk[cols_qk][None, :]),
            "bv": to_bf16(bv[cols_v][None, :]),
            "ones_row": to_bf16(np.ones((1, SB), np.float32)),
            "onesc": to_bf16(np.ones((128, DK), np.float32)),
            "onesr": np.ones((128, DK), np.float32),
            "maskA": _CAUSAL_MASKS[0],
            "maskB": _CAUSAL_MASKS[1],
            "cosT": cosT,
            "sinT": sinT,
        })
    return in_maps


# ---------------------------------------------------------------------------
# device program
# ---------------------------------------------------------------------------

def build_program(with_bias=False):
    from concourse import bacc, library_config
    nc = bacc.Bacc("TRN2", debug=False)

    xT = nc.declare_dram_parameter("xT", [D, S], BF16, isOutput=False).ap()
    wq = nc.declare_dram_parameter("wq", [D, NCOLS], BF16, isOutput=False).ap()
    wk = nc.declare_dram_parameter("wk", [D, NCOLS], BF16, isOutput=False).ap()
    wv = nc.declare_dram_parameter("wv", [D, NCOLS], BF16, isOutput=False).ap()
    wo = nc.declare_dram_parameter("wo", [DK, HG, D], BF16, isOutput=False).ap()
    bq = nc.declare_dram_parameter("bq", [1, NCOLS], BF16, isOutput=False).ap()
    bk = nc.declare_dram_parameter("bk", [1, NCOLS], BF16, isOutput=False).ap()
    bv = nc.declare_dram_parameter("bv", [1, NCOLS], BF16, isOutput=False).ap()
    ones_row_d = nc.declare_dram_parameter("ones_row", [1, SB], BF16, isOutput=False).ap()
    onesc_d = nc.declare_dram_parameter("onesc", [128, DK], BF16, isOutput=False).ap()
    onesr_d = nc.declare_dram_parameter("onesr", [128, DK], F32R, isOutput=False).ap()
    maskA_d = nc.declare_dram_parameter("maskA", [128, GW * SB], BF16, isOutput=False).ap()
    maskB_d = nc.declare_dram_parameter("maskB", [128, GW * SB], BF16, isOutput=False).ap()
    cosT = nc.declare_dram_parameter("cosT", [128, S], F32, isOutput=False).ap()
    sinT = nc.declare_dram_parameter("sinT", [128, S], F32, isOutput=False).ap()
    out = nc.declare_dram_parameter("out", [S, D], F32, isOutput=True).ap()
    debug_dump = os.environ.get("KERNEL_DEBUG_DUMP", "0") == "1"
    if debug_dump:
        dbg_qt = nc.declare_dram_parameter("dbg_qt", [128, SB], F32, isOutput=True).ap()
        dbg_kh = nc.declare_dram_parameter("dbg_kh", [128, SB], F32, isOutput=True).ap()
        dbg_va = nc.declare_dram_parameter("dbg_va", [128, HG * (DK + 8)], F32, isOutput=True).ap()
        dbg_ot = nc.declare_dram_parameter("dbg_ot", [NSB, HG, DK, SB], F32, isOutput=True).ap()
        dbg_den = nc.declare_dram_parameter("dbg_den", [NSB, HG, 2, SB], F32, isOutput=True).ap()

    with tile.TileContext(nc) as tc, ExitStack() as ctx:
        nc.gpsimd.load_library(library_config.proxy)
        const = ctx.enter_context(tc.tile_pool(name="const", bufs=1))
        sbig = ctx.enter_context(tc.tile_pool(name="sbig", bufs=1))
        rtmp = ctx.enter_context(tc.tile_pool(name="rtmp", bufs=2))
        epool = ctx.enter_context(tc.tile_pool(name="epool", bufs=3))
        npool = ctx.enter_context(tc.tile_pool(name="npool", bufs=2))
        opool = ctx.enter_context(tc.tile_pool(name="opool", bufs=2))
        sc_ps = ctx.enter_context(tc.tile_pool(name="sc_ps", bufs=2, space="PSUM"))
        pv_ps = ctx.enter_context(tc.tile_pool(name="pv_ps", bufs=2, space="PSUM"))
        mm_ps = ctx.enter_context(tc.tile_pool(name="mm_ps", bufs=2, space="PSUM"))

        # --- static SBUF tiles
        wq_sb = [const.tile([128, NCOLS], BF16, tag=f"wq{dc}", name=f"wq{dc}")
                 for dc in range(NDC)]
        wk_sb = [const.tile([128, NCOLS], BF16, tag=f"wk{dc}", name=f"wk{dc}")
                 for dc in range(NDC)]
        wv_sb = [const.tile([128, NCOLS], BF16, tag=f"wv{dc}", name=f"wv{dc}")
                 for dc in range(NDC)]
        cos_sb = const.tile([128, S], F32, tag="cos")
        sin_sb = const.tile([128, S], F32, tag="sin")
        wo_sb = const.tile([DK, HG, D], BF16, tag="wo")
        onesc_sb = const.tile([128, DK], BF16, tag="onesc")
        onesr_sb = const.tile([128, DK], F32R, tag="onesr")
        mask_sb = [const.tile([128, GW * SB], BF16, tag=f"mask{i}", name=f"mask{i}")
                   for i in range(2)]
        if with_bias:
            bq_sb = const.tile([1, NCOLS], BF16, tag="bq")
            bk_sb = const.tile([1, NCOLS], BF16, tag="bk")
            bv_sb = const.tile([1, NCOLS], BF16, tag="bv")
            ones_row = const.tile([1, SB], BF16, tag="ones_row")
        xt = [[sbig.tile([128, SB], BF16, tag=f"xt{sb}_{dc}", name=f"xt{sb}_{dc}")
               for dc in range(NDC)] for sb in range(NSB)]
        # Q^T / K^T per (chunk, sq-block): chunk c holds heads {2c, 2c+1}
        # stacked on partitions (head 2c rows 0:64, head 2c+1 rows 64:128)
        qt = [[sbig.tile([128, SB], BF16, tag=f"qt{c}_{sb}", name=f"qt{c}_{sb}")
               for sb in range(NSB)] for c in range(2)]
        kth = [[sbig.tile([128, SB], BF16, tag=f"kh{c}_{sb}", name=f"kh{c}_{sb}")
                for sb in range(NSB)] for c in range(2)]
        # V augmented per key tile, every head [V(64) | one] so PV row 64
        # accumulates the softmax denominator
        vaug = [sbig.tile([128, HG * AUGW], BF16, tag=f"va{st}", name=f"va{st}")
                for st in range(NST)]
        # normalized O^T per (head, sq-block), rows 0:64
        ot = [[sbig.tile([DK, SB], BF16, tag=f"ot{h}_{j}", name=f"ot{h}_{j}")
               for j in range(NSB)] for h in range(HG)]

        # --- DMAs, critical-path first
        for dc in range(NDC):
            nc.sync.dma_start(wq_sb[dc][:], wq[128 * dc:128 * dc + 128, :])
            nc.sync.dma_start(xt[0][dc][:], xT[128 * dc:128 * dc + 128, 0:SB])
        nc.sync.dma_start(cos_sb[:], cosT)
        nc.sync.dma_start(sin_sb[:], sinT)
        for dc in range(NDC):
            nc.sync.dma_start(wk_sb[dc][:], wk[128 * dc:128 * dc + 128, :])
        for dc in range(NDC):
            nc.sync.dma_start(wv_sb[dc][:], wv[128 * dc:128 * dc + 128, :])
        nc.sync.dma_start(onesc_sb[:], onesc_d)
        nc.sync.dma_start(onesr_sb[:], onesr_d)
        nc.sync.dma_start(mask_sb[0][:], maskA_d)
        nc.sync.dma_start(mask_sb[1][:], maskB_d)
        for dc in range(NDC):
            nc.sync.dma_start(xt[1][dc][:], xT[128 * dc:128 * dc + 128, SB:2 * SB])
        nc.sync.dma_start(wo_sb[:], wo)
        if with_bias:
            nc.sync.dma_start(bq_sb[:], bq)
            nc.sync.dma_start(bk_sb[:], bk)
            nc.sync.dma_start(bv_sb[:], bv)
            nc.sync.dma_start(ones_row[:], ones_row_d)
        for sb in (2, 3):
            for dc in range(NDC):
                nc.sync.dma_start(xt[sb][dc][:],
                                  xT[128 * dc:128 * dc + 128, SB * sb:SB * sb + SB])

        # --- emission helpers -------------------------------------------
        def emit_qk_chunk(sb, c, wname):
            """Projection chunk c of Q or K for sq block sb, incl. RoPE."""
            w_sb = wq_sb if wname == "q" else wk_sb
            ss = slice(SB * sb, SB * sb + SB)
            ncol = slice(128 * c, 128 * c + 128)
            ps = mm_ps.tile([128, SB], F32, tag="mm", name="ps_qk")
            for dc in range(NDC):
                nc.tensor.matmul(ps[:], w_sb[dc][:, ncol], xt[sb][dc][:],
                                 start=(dc == 0),
                                 stop=(dc == NDC - 1 and not with_bias))
            if with_bias:
                b_sb = bq_sb if wname == "q" else bk_sb
                nc.tensor.matmul(ps[:], b_sb[0:1, ncol], ones_row[0:1, :],
                                 start=False, stop=True)
            # rope: dst = ps*cos + shuffle(ps)*sin
            t_cos = rtmp.tile([128, SB], F32, tag="rc", name="t_cos")
            nc.vector.tensor_mul(t_cos[:], ps[:], cos_sb[:, ss])
            t_shuf = rtmp.tile([128, SB], F32, tag="rs", name="t_shuf")
            nc.vector.stream_shuffle(t_shuf[:], ps[:], SHUF_MASK)
            t_sin = rtmp.tile([128, SB], F32, tag="rm", name="t_sin")
            nc.gpsimd.tensor_mul(t_sin[:], t_shuf[:], sin_sb[:, ss])
            dst = qt[c][sb] if wname == "q" else kth[c][sb]
            nc.vector.tensor_add(dst[:], t_cos[:], t_sin[:])

        def emit_v_st(sb, st4):
            """V projection for one 128-seq tile, scattered into vaug."""
            st = 4 * sb + st4
            ps = mm_ps.tile([128, SB], F32, tag="mm", name="ps_v")
            for dc in range(NDC):
                nc.tensor.matmul(ps[:, 0:NCOLS],
                                 xt[sb][dc][:, 128 * st4:128 * st4 + 128],
                                 wv_sb[dc][:],
                                 start=(dc == 0),
                                 stop=(dc == NDC - 1 and not with_bias))
            if with_bias:
                nc.tensor.matmul(ps[:, 0:NCOLS], ones_row[0:1, 0:128],
                                 bv_sb[0:1, :], start=False, stop=True)
            va = vaug[st][:].rearrange("p (h e) -> p h e", h=HG)
            psv = ps[:, 0:NCOLS].rearrange("p (h k) -> p h k", h=HG)
            nc.vector.tensor_copy(va[:, :, 0:DK], psv[:, :, :])
            nc.vector.tensor_copy(va[:, :, DK], onesc_sb[:, 0:HG])

        wo_copy_tick = [0]

        def emit_wo(st, dc):
            """Output projection for one (128-seq, 512-dmodel) tile."""
            jb = st // 4
            rq = slice(128 * (st % 4), 128 * (st % 4) + 128)
            cols = slice(SB * dc, SB * dc + SB)
            ps = mm_ps.tile([128, SB], F32, tag="mm", name="ps_wo")
            for h in range(HG):
                nc.tensor.matmul(ps[:], ot[h][jb][:, rq], wo_sb[:, h, cols],
                                 start=(h == 0), stop=(h == HG - 1))
            o_sb = opool.tile([128, SB], F32, tag="osb", name="o_sb")
            if wo_copy_tick[0] % 2 == 0:
                nc.vector.tensor_copy(o_sb[:], ps[:])
            else:
                nc.scalar.copy(o_sb[:], ps[:])
            wo_copy_tick[0] += 1
            nc.sync.dma_start(out[128 * st:128 * st + 128, cols], o_sb[:])

        def emit_norm(pv_t, rec, h, j):
            """ot[h][j] = pv V-rows * broadcast(1/denominator)."""
            bcp = mm_ps.tile([128, SB], F32, tag="mm", name="bcp")
            nc.tensor.matmul(bcp[0:DK, :], onesr_sb[DK:DK + 1, :],
                             rec[DK:DK + 1, :],
                             start=True, stop=True)
            bc = npool.tile([128, SB], F32, tag="bcs", name="bc")
            nc.vector.tensor_copy(bc[0:DK, :], bcp[0:DK, :])
            nc.vector.tensor_mul(ot[h][j][:], pv_t[0:DK, :], bc[0:DK, :])

        # --- projections for block 0 (nothing to overlap them with)
        emit_qk_chunk(0, 0, "q")
        emit_qk_chunk(0, 0, "k")
        emit_qk_chunk(0, 1, "q")
        emit_qk_chunk(0, 1, "k")
        for st4 in range(4):
            emit_v_st(0, st4)

        # --- main interleaved stream ------------------------------------
        seq = os.environ.get("KERNEL_SEQ", "0") == "1"
        if seq:
            for nb in range(1, NSB):
                emit_qk_chunk(nb, 0, "q")
                emit_qk_chunk(nb, 0, "k")
                emit_qk_chunk(nb, 1, "q")
                emit_qk_chunk(nb, 1, "k")
                for st4 in range(4):
                    emit_v_st(nb, st4)
        fillers = deque()
        pending_norm = deque()  # closures, flushed after the next sc group
        for j in range(NSB):
            if not seq and j < NSB - 1:
                nb = j + 1
                fillers.append(lambda nb=nb: emit_qk_chunk(nb, 0, "q"))
                fillers.append(lambda nb=nb: emit_qk_chunk(nb, 0, "k"))
                fillers.append(lambda nb=nb: emit_qk_chunk(nb, 1, "q"))
                fillers.append(lambda nb=nb: emit_qk_chunk(nb, 1, "k"))
                for st4 in range(4):
                    fillers.append(lambda nb=nb, st4=st4: emit_v_st(nb, st4))
            if not seq and j > 0:
                for st in range(4 * (j - 1), 4 * j):
                    for dc in range(2):
                        fillers.append(lambda st=st, dc=dc: emit_wo(st, dc))

            for h in range(HG):
                c, parity = divmod(h, 2)
                rows = slice(DK * parity, DK * parity + DK)
                pv = pv_ps.tile([128, SB], F32, tag="pv", name="pv")
                pv_out = pv[0:DK + 1, :]
                ngrp = 2 * (j + 1)
                for g in range(ngrp):
                    sc = sc_ps.tile([128, GW * SB], F32, tag="sc", name="sc")
                    for t in range(GW):
                        i = GW * g + t
                        nc.tensor.matmul(
                            sc[:, SB * t:SB * t + SB],
                            kth[c][i // 4][rows, 128 * (i % 4):128 * (i % 4) + 128],
                            qt[c][j][rows, :],
                            start=True, stop=True)
                    while pending_norm:
                        pending_norm.popleft()()
                    e = epool.tile([128, GW * SB], BF16, tag="e", name="e")
                    nc.scalar.activation(e[:], sc[:],
                                         mybir.ActivationFunctionType.Exp,
                                         scale=SCALE)
                    d0 = GW * g - 4 * j
                    if d0 + GW > 0:  # group touches the causal diagonal
                        nc.vector.tensor_mul(e[:], e[:],
                                             mask_sb[d0 // 2][:])
                    if fillers:
                        fillers.popleft()()
                    for t in range(GW):
                        i = GW * g + t
                        lhs = vaug[i][:].rearrange("p (h e) -> p h e", h=HG)[:, h, 0:DK + 1]
                        nc.tensor.matmul(
                            pv_out, lhs, e[:, SB * t:SB * t + SB],
                            start=(g == 0 and t == 0),
                            stop=(g == ngrp - 1 and t == GW - 1))
                # denominator reciprocal now; broadcast+normalize deferred so
                # the recip latency hides under the next head's first group
                rec = npool.tile([128, SB], F32R, tag="rec", name="rec")
                lnt = npool.tile([128, SB], F32, tag="lnt", name="lnt")
                if debug_dump:
                    dd = opool.tile([128, SB], F32, tag="dbg", name="dd")
                    nc.vector.tensor_copy(dd[DK:DK + 1, :], pv[DK:DK + 1, :])
                    nc.sync.dma_start(dbg_den[j, h, 0], dd[DK:DK + 1, :])
                # 1/denom via exp(-ln(denom)) on the Scalar engine: two
                # ~0.5us table ops instead of a 3.3us DVE reciprocal that
                # would head-block the in-order Vector queue
                nc.scalar.activation(lnt[DK:DK + 1, :], pv[DK:DK + 1, :],
                                     mybir.ActivationFunctionType.Ln)
                nc.scalar.activation(rec[DK:DK + 1, :], lnt[DK:DK + 1, :],
                                     mybir.ActivationFunctionType.Exp,
                                     scale=-1.0)
                if debug_dump:
                    nc.sync.dma_start(dbg_den[j, h, 1], rec[DK:DK + 1, :])
                pending_norm.append(
                    lambda pv_t=pv, rec=rec, hh=h, jj=j:
                        emit_norm(pv_t, rec, hh, jj))

        # --- tail: last normalize + Wo for block 3
        while pending_norm:
            pending_norm.popleft()()
        while fillers:
            fillers.popleft()()
        wo_start = 0 if seq else 4 * (NSB - 1)
        for st in range(wo_start, 4 * NSB):
            for dc in range(2):
                emit_wo(st, dc)

        if debug_dump:
            dq = opool.tile([128, SB], F32, tag="dbg", name="dq")
            nc.vector.tensor_copy(dq[:], qt[0][1][:])
            nc.sync.dma_start(dbg_qt, dq[:])
            dk_ = opool.tile([128, SB], F32, tag="dbg", name="dk_")
            nc.vector.tensor_copy(dk_[:], kth[0][1][:])
            nc.sync.dma_start(dbg_kh, dk_[:])
            dv = opool.tile([128, HG * AUGW], F32, tag="dbgv", name="dv")
            nc.vector.tensor_copy(dv[:], vaug[4][:])
            nc.sync.dma_start(dbg_va, dv[:])
            for jj in range(NSB):
                for hh in range(HG):
                    do = opool.tile([128, SB], F32, tag="dbg", name="do")
                    nc.vector.tensor_copy(do[0:DK, :], ot[hh][jj][:])
                    nc.sync.dma_start(dbg_ot[jj, hh], do[0:DK, :])

    nc.compile()
    return nc


_CACHED_NC = {}


def _get_program(with_bias=False):
    if with_bias not in _CACHED_NC:
        _CACHED_NC[with_bias] = build_program(with_bias=with_bias)
    return _CACHED_NC[with_bias]


# ---------------------------------------------------------------------------
# entry point
# ---------------------------------------------------------------------------

def kernel(x, token_position, Wq, bq, Wk, bk, Wv, bv, Wo, bo, _results=None):
    from concourse.bass_utils import run_bass_kernel_spmd

    in_maps = make_core_inputs(x, token_position, Wq, bq, Wk, bk, Wv, bv, Wo, bo)
    if _results is None:
        with_bias = any(float(np.abs(np.asarray(v)).max()) != 0.0
                        for v in (bq, bk, bv))
        nc = _get_program(with_bias=with_bias)
        res = run_bass_kernel_spmd(nc, in_maps, list(range(N_CORES)))
        _results = [res.results[i]["out"] for i in range(N_CORES)]
    bo = np.asarray(bo, dtype=np.float32)
    out = np.empty((B, S, D), dtype=np.float32)
    for b in range(B):
        acc = _results[HG * b].astype(np.float32)
        for hg in range(1, HG):
            acc = acc + _results[HG * b + hg]
        out[b] = acc + bo[None, :]
    return out
